# revision 2
# baseline (speedup 1.0000x reference)
"""CeNN front-end Trainium2 kernel, v2: fully SBUF-resident iteration.

Reference (per batch image u [1,H,W], H=W=512, 64 ch, 16 steps):
    control = conv3x3_same(u, W_B)
    x0 = control
    x_{k+1} = alpha*x_k + beta*(conv3x3_same(tanh(x_k), WA_eff) + control + bias)

Distribution: 8 cores each own 64 image rows (H split 8 ways); the 4
batches run as sequential rounds on every core.  Per round a core holds a
96-row window (own rows + 16-row halo each side) entirely in SBUF:
no DRAM round-trips between steps (baseline streamed 1 GB/core through
HBM; this streams ~35 MB).

Trapezoid: at step k only window rows [k, 96-k) are updated — the halo
shrinks by one row per side per step, ending exactly at the 64 own rows.
Uniform SPMD program; image-edge boundary condition is enforced by a
per-core mask (0 for the out-of-image side) multiplied into the tanh of
window row 15 (image row -1 for core 0) and row 80 (image row 512 for
core 7) each step, which restores the reference's zero padding at the
only rows where it matters.

Layout: x [128 part, 48, 514] fp32 — channels 0-63 on partitions 0:64
hold window rows 0-47 (block A), partitions 64:128 hold rows 48-95
(block B); cols 0/513 are zero pads.  C = beta*(control+bias) resident
bf16 [128, 48, 512].  conv3x3 = 9 accumulating bf16 matmuls (K=64 cin,
M=64 cout, N=512) at per-tap free offsets + 1 identity tap adding C.
Four PE quadrants (tile_position) process 4 rows concurrently.  Update
is one fused DVE op per psum tile: x' = x*alpha + psum.

Pass 0 per round computes control with a K=10 bf16 im2col matmul
(9 shifted u rows + ones row folding in bias), two rows per psum via
column tiling.
"""

import math

import numpy as np
import ml_dtypes

import concourse.bacc as bacc
import concourse.tile as tile
from concourse import mybir
from concourse.bass_utils import run_bass_kernel_spmd

F32 = mybir.dt.float32
BF16 = mybir.dt.bfloat16
AF = mybir.ActivationFunctionType
ALU = mybir.AluOpType

NR = 4            # batch rounds
RH = 48           # rows per partition block; window = 2*RH = 96
NSTEPS = 16
RC = 6            # pass-0 chunk rows (per block)
WP = 514
W = 512


def build():
    nc = bacc.Bacc("TRN2", target_bir_lowering=False, debug=False,
                   num_devices=8)

    u_in = nc.dram_tensor("u_in", [NR, 98, W], BF16, kind="ExternalInput")
    wa_in = nc.dram_tensor("wa_in", [64, 10, 64], BF16, kind="ExternalInput")
    wb_in = nc.dram_tensor("wb_in", [10, 64], BF16, kind="ExternalInput")
    nbias_in = nc.dram_tensor("nbias_in", [64, 1], F32, kind="ExternalInput")
    alpha_in = nc.dram_tensor("alpha_in", [1, 1], F32, kind="ExternalInput")
    mtop_in = nc.dram_tensor("mtop_in", [1, 1], F32, kind="ExternalInput")
    mbot_in = nc.dram_tensor("mbot_in", [1, 1], F32, kind="ExternalInput")
    x_out = nc.dram_tensor("x_out", [NR, 64, 64, W], F32,
                           kind="ExternalOutput")

    with tile.TileContext(nc) as tc:
        with tc.tile_pool(name="singles", bufs=1) as singles:
            wa_t = singles.tile([128, 10, 64], BF16)
            nc.sync.dma_start(out=wa_t[0:64], in_=wa_in[:, :, :])
            nc.sync.dma_start(out=wa_t[64:128], in_=wa_in[:, :, :])
            wb_t = singles.tile([10, 64], BF16)
            nc.sync.dma_start(out=wb_t, in_=wb_in[:, :])
            nbias_t = singles.tile([128, 1], F32)
            nc.sync.dma_start(out=nbias_t[0:64], in_=nbias_in[:, :])
            nc.sync.dma_start(out=nbias_t[64:128], in_=nbias_in[:, :])
            alpha_t = singles.tile([128, 1], F32)
            nc.sync.dma_start(out=alpha_t,
                              in_=alpha_in[:, :].to_broadcast((128, 1)))
            beta_t = singles.tile([128, 1], F32)
            nc.vector.tensor_scalar(out=beta_t, in0=alpha_t, scalar1=-1.0,
                                    scalar2=1.0, op0=ALU.mult, op1=ALU.add)
            mtop_t = singles.tile([128, 1], F32)
            nc.sync.dma_start(out=mtop_t,
                              in_=mtop_in[:, :].to_broadcast((128, 1)))
            mbot_t = singles.tile([128, 1], F32)
            nc.sync.dma_start(out=mbot_t,
                              in_=mbot_in[:, :].to_broadcast((128, 1)))

            with tc.tile_pool(name="xs", bufs=1) as xpool, \
                 tc.tile_pool(name="cc", bufs=1) as cpool, \
                 tc.tile_pool(name="u9", bufs=2) as upool, \
                 tc.tile_pool(name="th", bufs=16) as thpool, \
                 tc.tile_pool(name="thseam", bufs=4) as tspool, \
                 tc.tile_pool(name="ps", bufs=4, space="PSUM") as pspool, \
                 tc.tile_pool(name="pf", bufs=2, space="PSUM") as pfpool, \
                 tc.tile_pool(name="p0ps", bufs=2, space="PSUM") as p0pool:
                xs = xpool.tile([128, RH, WP], F32)
                cc = cpool.tile([128, RH, W], BF16)
                nc.vector.memset(xs[:, :, 0:1], 0.0)
                nc.vector.memset(xs[:, :, 513:514], 0.0)

                n_chunk = 0
                for r in range(NR):
                    # ---------------- pass 0: control -> x0, C --------------
                    for c0 in range(0, RH, RC):
                        u9 = upool.tile([10, 2, RC, W], BF16)
                        if n_chunk < 2:
                            # pool buffers persist: zeros in the DMA-uncovered
                            # edge cols and the ones row survive reuse
                            nc.vector.memset(u9, 0.0)
                            nc.vector.memset(u9[0:1, :, :, :], 1.0)
                        n_chunk += 1
                        for t9 in range(9):
                            kh, kw = divmod(t9, 3)
                            c_lo = max(0, 1 - kw)
                            c_hi = min(W, W + 1 - kw)
                            for h in range(2):
                                nc.sync.dma_start(
                                    out=u9[t9 + 1:t9 + 2, h, 0:RC,
                                           c_lo:c_hi],
                                    in_=u_in[r,
                                             h * RH + c0 + kh:
                                             h * RH + c0 + kh + RC,
                                             c_lo + kw - 1:c_hi + kw - 1])
                        for t in range(RC):
                            pc = p0pool.tile([128, W], F32)
                            nc.tensor.matmul(pc[0:64], wb_t[0:10, :],
                                             u9[:, 0, t, :],
                                             start=True, stop=True,
                                             tile_position=(0, 0))
                            nc.tensor.matmul(pc[64:128], wb_t[0:10, :],
                                             u9[:, 1, t, :],
                                             start=True, stop=True,
                                             tile_position=(0, 64))
                            nc.scalar.activation(out=xs[:, c0 + t, 1:513],
                                                 in_=pc, func=AF.Identity,
                                                 bias=nbias_t, scale=1.0)
                            nc.scalar.activation(out=cc[:, c0 + t, :],
                                                 in_=pc, func=AF.Copy,
                                                 scale=beta_t)

                    # ---------------- steps 1..16 ---------------------------
                    for k in range(1, NSTEPS + 1):
                        lo, hi = k, 96 - k

                        th = [None] * RH

                        def mk_th(j):
                            if th[j] is not None:
                                return
                            pool = tspool if j in (0, RH - 1) else thpool
                            tt = pool.tile([128, WP], BF16)
                            nc.scalar.activation(out=tt, in_=xs[:, j, :],
                                                 func=AF.Tanh)
                            if j == 15:
                                nc.vector.scalar_tensor_tensor(
                                    out=tt[0:64, :], in0=tt[0:64, :],
                                    scalar=mtop_t[0:64], in1=tt[0:64, :],
                                    op0=ALU.mult, op1=ALU.bypass)
                            if j == 32:
                                nc.vector.scalar_tensor_tensor(
                                    out=tt[64:128, :], in0=tt[64:128, :],
                                    scalar=mbot_t[64:128], in1=tt[64:128, :],
                                    op0=ALU.mult, op1=ALU.bypass)
                            th[j] = tt

                        mk_th(0)
                        mk_th(RH - 1)

                        def row_taps(trow, ph, ps_tile, pf_tile):
                            """Matmul kwarg list for one row: 9 conv taps +
                            C tap in the main (own-block) PSUM group; cross-
                            block taps in a separate foreign group."""
                            dblk, dj = divmod(trow, RH)
                            dp = slice(dblk * 64, dblk * 64 + 64)
                            out_ps = ps_tile[ph * 64:ph * 64 + 64, :]
                            main, foreign = [], []
                            for t9 in range(9):
                                kh, kw = divmod(t9, 3)
                                srow = trow + kh - 1
                                sblk, sj = divmod(srow, RH)
                                (main if sblk == dblk else foreign).append(
                                    (t9, sblk, sj, kw))
                            ops = []
                            for i, (t9, sblk, sj, kw) in enumerate(main):
                                ops.append(dict(
                                    out=out_ps, lhsT=wa_t[dp, t9, :],
                                    rhs=th[sj][dp, kw:kw + 512],
                                    start=(i == 0), stop=False,
                                    tile_position=(dblk * 64, ph * 64)))
                            ops.append(dict(
                                out=out_ps, lhsT=wa_t[dp, 9, :],
                                rhs=cc[dp, dj, :],
                                start=False, stop=True,
                                tile_position=(dblk * 64, ph * 64)))
                            if foreign:
                                sblk = foreign[0][1]
                                sp = slice(sblk * 64, sblk * 64 + 64)
                                out_pf = pf_tile[ph * 64:ph * 64 + 64, :]
                                for i, (t9, _, sj, kw) in enumerate(foreign):
                                    ops.append(dict(
                                        out=out_pf, lhsT=wa_t[sp, t9, :],
                                        rhs=th[sj][sp, kw:kw + 512],
                                        start=(i == 0),
                                        stop=(i == len(foreign) - 1),
                                        tile_position=(sblk * 64, ph * 64)))
                            return ops, bool(foreign)

                        def upd(prow, prange, in1):
                            nc.vector.scalar_tensor_tensor(
                                out=xs[prange, prow, 1:513],
                                in0=xs[prange, prow, 1:513],
                                scalar=alpha_t[prange], in1=in1,
                                op0=ALU.mult, op1=ALU.add)

                        def fadd(prow, prange, in1):
                            nc.vector.scalar_tensor_tensor(
                                out=xs[prange, prow, 1:513],
                                in0=xs[prange, prow, 1:513],
                                scalar=1.0, in1=in1,
                                op0=ALU.bypass, op1=ALU.add)

                        def act(trow):
                            return lo <= trow < hi

                        for j0 in range(0, RH, 2):
                            j1 = j0 + 1
                            for j in range(max(0, j0 - 1),
                                           min(RH, j0 + 3)):
                                mk_th(j)
                            a0, b0 = act(j0), act(RH + j0)
                            if j1 < RH:
                                a1, b1 = act(j1), act(RH + j1)
                            else:
                                a1 = b1 = False
                            P0 = P1 = PF = None
                            need_pf = ((a0 and j0 == RH - 1)
                                       or (a1 and j1 == RH - 1)
                                       or (b0 and j0 == 0))
                            if need_pf:
                                PF = pfpool.tile([128, 512], F32)
                            if a0 or b0:
                                P0 = pspool.tile([128, 512], F32, tag="P")
                            if a1 or b1:
                                P1 = pspool.tile([128, 512], F32, tag="P")
                            seqs = []
                            frows = []
                            if a0:
                                s, f = row_taps(j0, 0, P0, PF)
                                seqs.append(s)
                                if f:
                                    frows.append((j0, slice(0, 64), 0))
                            if b0:
                                s, f = row_taps(RH + j0, 1, P0, PF)
                                seqs.append(s)
                                if f:
                                    frows.append((j0, slice(64, 128), 1))
                            if a1:
                                s, f = row_taps(j1, 1, P1, PF)
                                seqs.append(s)
                                if f:
                                    frows.append((j1, slice(0, 64), 1))
                            if b1:
                                s, f = row_taps(RH + j1, 0, P1, PF)
                                seqs.append(s)
                                if f:
                                    frows.append((j1, slice(64, 128), 0))
                            nmax = max((len(s) for s in seqs), default=0)
                            for t in range(nmax):
                                for s in seqs:
                                    if t < len(s):
                                        nc.tensor.matmul(
                                            s[t]["out"], s[t]["lhsT"],
                                            s[t]["rhs"],
                                            start=s[t]["start"],
                                            stop=s[t]["stop"],
                                            skip_group_check=True,
                                            tile_position=s[t][
                                                "tile_position"])
                            if a0 and b0:
                                upd(j0, slice(0, 128), P0)
                            else:
                                if a0:
                                    upd(j0, slice(0, 64), P0[0:64, :])
                                if b0:
                                    upd(j0, slice(64, 128), P0[64:128, :])
                            if a1:
                                upd(j1, slice(0, 64), P1[64:128, :])
                            if b1:
                                upd(j1, slice(64, 128), P1[0:64, :])
                            for (pj, xsl, phh) in frows:
                                fadd(pj, xsl, PF[phh * 64:phh * 64 + 64, :])

                    # ---------------- store own rows ------------------------
                    nc.sync.dma_start(out=x_out[r, :, 0:32, :],
                                      in_=xs[0:64, 16:48, 1:513])
                    nc.sync.dma_start(out=x_out[r, :, 32:64, :],
                                      in_=xs[64:128, 0:32, 1:513])

    nc.compile()
    return nc


def host_prep(u, W_B, W_A, bias, alpha_logit):
    """Per-core input maps."""
    B = u.shape[0]
    H = u.shape[2]

    alpha = np.float32(1.0 / (1.0 + np.exp(-np.float64(alpha_logit))))
    beta = np.float32(1.0) - alpha

    WAe = np.array(W_A, dtype=np.float32).copy()
    idx = np.arange(64)
    WAe[idx, idx, 1, 1] = np.maximum(WAe[idx, idx, 1, 1], np.float32(1.0))

    wa_taps = np.zeros((64, 10, 64), dtype=np.float32)
    for t9 in range(9):
        kh, kw = divmod(t9, 3)
        wa_taps[:, t9, :] = (beta * WAe[:, :, kh, kw]).T   # [cin, cout]
    wa_taps[:, 9, :] = np.eye(64, dtype=np.float32)
    wa_taps = wa_taps.astype(ml_dtypes.bfloat16)

    bias_vec = np.array(bias, dtype=np.float32).reshape(64)
    wb10 = np.zeros((10, 64), dtype=np.float32)
    wb10[0, :] = bias_vec
    for t9 in range(9):
        kh, kw = divmod(t9, 3)
        wb10[t9 + 1, :] = W_B[:, 0, kh, kw]
    wb10 = wb10.astype(ml_dtypes.bfloat16)
    nbias = (-bias_vec).reshape(64, 1).astype(np.float32)
    alpha_arr = np.full((1, 1), alpha, dtype=np.float32)

    u_f = np.asarray(u, dtype=np.float32)
    in_maps = []
    for core in range(8):
        w0 = 64 * core - 16
        u_slab = np.zeros((NR, 98, W), dtype=np.float32)
        lo = max(0, w0 - 1)
        hi = min(H, w0 + 97)
        u_slab[:, lo - (w0 - 1):hi - (w0 - 1), :] = u_f[:, 0, lo:hi, :]
        in_maps.append({
            "u_in": u_slab.astype(ml_dtypes.bfloat16),
            "wa_in": wa_taps,
            "wb_in": wb10,
            "nbias_in": nbias,
            "alpha_in": alpha_arr,
            "mtop_in": np.full((1, 1), 0.0 if core == 0 else 1.0,
                               dtype=np.float32),
            "mbot_in": np.full((1, 1), 0.0 if core == 7 else 1.0,
                               dtype=np.float32),
        })
    return in_maps


_NC_CACHE = {}


def _get_nc():
    if "nc" not in _NC_CACHE:
        _NC_CACHE["nc"] = build()
    return _NC_CACHE["nc"]


def kernel(u, W_B, W_A, bias, alpha_logit, _trace=False):
    u = np.asarray(u, dtype=np.float32)
    B, _, H, Wc = u.shape
    nc = _get_nc()
    in_maps = host_prep(u, W_B, W_A, bias, alpha_logit)
    res = run_bass_kernel_spmd(nc, in_maps, core_ids=list(range(8)),
                               trace=_trace)
    out = np.zeros((B, 64, H, Wc), dtype=np.float32)
    for core in range(8):
        xo = res.results[core]["x_out"]          # [NR, 64, 64, 512]
        out[:, :, 64 * core:64 * core + 64, :] = xo
    kernel._last_results = res
    return out
